# revision 13
# baseline (speedup 1.0000x reference)
"""Trainium2 Bass kernel for nn_Log_GraphConv4d (log-shift-max + 1x1 conv + BN + GeLU).

Math refactor (validated in numpy):
  reference x_j = max_s max(0, x - roll(x, s)) over s in {±1,±3,±7,±15,±31} on H and W
            == x - m,  where m = min(x, all 20 rolls)
  y = W1 @ x + W2 @ x_j            (1x1 conv, channel concat [x, x_j], K=768)
    = (W1+W2) @ x + (-W2) @ m
  BN (eval) + conv bias fold into per-out-channel affine (a, b):
  out = gelu(a * (Wc @ [x; m]) + b)

The min over the 20 rolls + x is computed with a 14-op binary min DAG
(proven minimal by exhaustive search over union-of-translates DAGs: each
axis needs build-ops + merge-leaves >= 7). All min ops run on DVE in bf16
2x mode; a helper copy X1 = roll(x, +1 in w) keeps W-axis reads 4B-aligned.

v2 changes vs baseline:
  - x is cast to bf16 on the host: input DMA traffic and fill latency halve.
  - X1 is built by GPSIMD tensor_copy (Pool engine is otherwise idle),
    freeing the Scalar engine for activations only.
  - y is written as bf16 and upcast on the host: drain shortens.
  - The final DAG op (min with x) is emitted per-C-chunk into 3 separate
    tiles so the K-contraction matmuls can start before the whole m tile
    is finished.

Sharding: data-parallel over batch, 2 samples per core across 8 cores;
weights replicated. No collectives.
"""

import numpy as np
import ml_dtypes

import concourse.bass as bass
import concourse.mybir as mybir
from concourse import bacc
from concourse.tile import TileContext
from concourse.bass_utils import run_bass_kernel_spmd

N_CORES = 8
B, C, H, W = 16, 384, 56, 56
HW = H * W                 # 3136
B_LOC = B // N_CORES       # 2 samples per core
CC = C // 128              # 3 input-channel chunks
OC = 384 // 128            # 3 output-channel chunks
NT = 7                     # n tiles over HW
NF = HW // NT              # 448 columns per matmul (fits one PSUM bank)

BF16 = mybir.dt.bfloat16
F32 = mybir.dt.float32
MIN = mybir.AluOpType.min
GELU = mybir.ActivationFunctionType.Gelu

LAST_RESULTS = None        # BassKernelResults of the most recent run (for test harness)


def _emit_min(eng, out, a, sa, b, sb, axis, L):
    """out = min(roll(a, sa, axis), roll(b, sb, axis)) on [128, cc, H, W] tiles.

    roll(t, s, axis)[i] = t[(i + s) mod L] along `axis` (2 = h, 3 = w).
    Circular wrap is handled by splitting into contiguous AP regions.
    """
    sa %= L
    sb %= L
    cuts = sorted({0, (L - sa) % L, (L - sb) % L})
    for idx, p in enumerate(cuts):
        q = cuts[idx + 1] if idx + 1 < len(cuts) else L
        n = q - p
        alo = (p + sa) % L
        blo = (p + sb) % L
        if axis == 3:
            eng.tensor_tensor(
                out=out[:, :, :, p:q],
                in0=a[:, :, :, alo:alo + n],
                in1=b[:, :, :, blo:blo + n],
                op=MIN,
            )
        else:
            eng.tensor_tensor(
                out=out[:, :, p:q, :],
                in0=a[:, :, alo:alo + n, :],
                in1=b[:, :, blo:blo + n, :],
                op=MIN,
            )


def _build(b_loc=B_LOC, cc=CC, oc=OC, act=GELU, repeat=1):
    nc = bacc.Bacc(None, target_bir_lowering=False)
    kc = 2 * cc

    xd = nc.dram_tensor("x_in", [b_loc, cc, 128, HW], BF16, kind="ExternalInput")
    wd = nc.dram_tensor("w_in", [kc, 128, oc * 128], BF16, kind="ExternalInput")
    pd = nc.dram_tensor("p_in", [oc, 128, 2], F32, kind="ExternalInput")
    yd = nc.dram_tensor("y_out", [b_loc, oc, 128, HW], BF16, kind="ExternalOutput")

    with TileContext(nc) as tc:
        with tc.tile_pool(name="sb", bufs=1) as pool, \
             tc.tile_pool(name="ps", bufs=6, space="PSUM") as psum:

            # --- prologue: weights + folded BN params (replicated, tiny) ---
            w_sb = pool.tile([128, kc, oc * 128], BF16, tag="w", bufs=1, name="w_sb")
            for k in range(kc):
                nc.sync.dma_start(out=w_sb[:, k, :], in_=wd[k])
            prm = pool.tile([128, oc, 2], F32, tag="prm", bufs=1, name="prm")
            nc.sync.dma_start(out=prm, in_=pd.rearrange("o p t -> p o t"))

            for b in [b for _ in range(repeat) for b in range(b_loc)]:
                # --- load bf16 x (cast on host), one DMA per C-chunk ---
                X = pool.tile([128, cc, H, W], BF16, tag="X", bufs=2, name="X")
                Xf3 = X.rearrange("p c h w -> p c (h w)")
                for c in range(cc):
                    for ph in range(0, 128, 32):
                        nc.sync.dma_start(
                            out=Xf3[ph:ph + 32, c],
                            in_=xd[b, c, ph:ph + 32],
                        )

                # X1 = roll(X, +1 in w) on Scalar, per C-chunk so it starts
                # as soon as each X chunk lands: keeps W-axis reads aligned.
                X1 = pool.tile([128, cc, H, W], BF16, tag="X1", bufs=1, name="X1")
                for c in range(cc):
                    nc.scalar.copy(out=X1[:, c, :, 0:W - 1], in_=X[:, c, :, 1:W])
                    nc.scalar.copy(out=X1[:, c, :, W - 1:W], in_=X[:, c, :, 0:1])

                # --- 14-op min DAG (offsets validated against reference) ---
                # H-chain (axis=2) on X directly (row stride is even):
                #   A2={±3} B2={1,±3,7} C2={±1,±3,±7} U2={15,31} V2={±15,±31}
                # W-chain (axis=3) via X1 so every shift is even; H-only ops
                # lead so the DVE starts before the X1 copy completes.
                # Merges M1/M2/M3/m run per C-chunk so matmuls start early.
                tA2 = pool.tile([128, cc, H, W], BF16, tag="t4", bufs=1, name="tA2")
                tU2 = pool.tile([128, cc, H, W], BF16, tag="t5", bufs=1, name="tU2")
                for c in range(cc):  # chunked: start on first-landed X chunk
                    _emit_min(nc.vector, tA2[:, c:c + 1], X[:, c:c + 1], 3,
                              X[:, c:c + 1], -3, 2, H)                     # A2
                for c in range(cc):
                    _emit_min(nc.vector, tU2[:, c:c + 1], X[:, c:c + 1], 15,
                              X[:, c:c + 1], 31, 2, H)                     # U2
                tA = pool.tile([128, cc, H, W], BF16, tag="t1", bufs=1, name="tA")
                _emit_min(nc.vector, tA, X1, 2, X1, -4, 3, W)              # A
                tB = pool.tile([128, cc, H, W], BF16, tag="t2", bufs=1, name="tB")
                _emit_min(nc.vector, tB, tA, 0, tA, 4, 3, W)               # B
                tB2 = pool.tile([128, cc, H, W], BF16, tag="t1", bufs=1, name="tB2")
                _emit_min(nc.vector, tB2, tA2, 0, tA2, 4, 2, H)            # B2
                tC = pool.tile([128, cc, H, W], BF16, tag="t3", bufs=1, name="tC")
                _emit_min(nc.vector, tC, tB, 0, tB, -4, 3, W)              # C
                tU = pool.tile([128, cc, H, W], BF16, tag="t2", bufs=1, name="tU")
                _emit_min(nc.vector, tU, X1, 14, X1, 30, 3, W)             # U
                tC2 = pool.tile([128, cc, H, W], BF16, tag="t4", bufs=1, name="tC2")
                _emit_min(nc.vector, tC2, tB2, 0, tB2, -4, 2, H)           # C2
                tV = pool.tile([128, cc, H, W], BF16, tag="t1", bufs=1, name="tV")
                _emit_min(nc.vector, tV, tU, 0, tU, -46, 3, W)             # V
                tV2 = pool.tile([128, cc, H, W], BF16, tag="t2", bufs=1, name="tV2")
                _emit_min(nc.vector, tV2, tU2, 0, tU2, -46, 2, H)          # V2

                # Chunked merge rounds: M1=min(C,V) M2=min(C2,V2)
                # M3=min(M1,M2) m=min(M3,X); M2/M3 write in place.
                mpc = []
                for c in range(cc):
                    tM1 = tU2  # t5: U2 fully consumed by V2 above
                    nc.vector.tensor_tensor(
                        out=tM1[:, c], in0=tC[:, c], in1=tV[:, c], op=MIN)
                    nc.vector.tensor_tensor(
                        out=tC2[:, c], in0=tC2[:, c], in1=tV2[:, c], op=MIN)
                    nc.vector.tensor_tensor(
                        out=tM1[:, c], in0=tM1[:, c], in1=tC2[:, c], op=MIN)
                    mp_c = pool.tile([128, H, W], BF16, tag=f"mp{c}", bufs=2,
                                     name=f"mp{c}")
                    nc.vector.tensor_tensor(
                        out=mp_c, in0=tM1[:, c], in1=X[:, c], op=MIN)
                    mpc.append(mp_c)

                # --- matmul K=2C contraction + fused BN-affine + GeLU ---
                Xf = X.rearrange("p c h w -> p c (h w)")
                Mfs = [m.rearrange("p h w -> p (h w)") for m in mpc]
                # k-phase emission with 7 PSUM tiles per o: all the
                # X-dependent accumulation (k<cc) issues before any
                # m-dependent matmul, so the PE queue never head-of-line
                # blocks on mp chunks mid-tile.
                for o in range(oc):
                    y_sb = pool.tile([128, HW], BF16, tag="y", bufs=2, name="y_sb")
                    psts = [psum.tile([128, NF], F32, tag="ps", bufs=7,
                                      name=f"pst{n}") for n in range(NT)]
                    for k in range(kc):
                        for n in range(NT):
                            if k < cc:
                                src = Xf[:, k, n * NF:(n + 1) * NF]
                            else:
                                src = Mfs[k - cc][:, n * NF:(n + 1) * NF]
                            nc.tensor.matmul(
                                psts[n],
                                lhsT=w_sb[:, k, o * 128:(o + 1) * 128],
                                rhs=src,
                                start=(k == 0),
                                stop=(k == kc - 1),
                            )
                    for n in range(NT):
                        nc.scalar.activation(
                            out=y_sb[:, n * NF:(n + 1) * NF],
                            in_=psts[n],
                            func=act,
                            bias=prm[:, o, 1:2],
                            scale=prm[:, o, 0:1],
                        )
                        nc.sync.dma_start(
                            out=yd[b, o, :, n * NF:(n + 1) * NF],
                            in_=y_sb[:, n * NF:(n + 1) * NF],
                        )
    nc.finalize()  # Bacc: wait-splitting, reg alloc, event sems — required by walrus
    return nc


_CACHE = {}


def _get_program():
    if "nc" not in _CACHE:
        _CACHE["nc"] = _build()
    return _CACHE["nc"]


def kernel(x, conv_w, conv_b, bn_scale, bn_bias, bn_mean, bn_var, _trace=False):
    global LAST_RESULTS
    x = np.asarray(x, dtype=np.float32)
    conv_w = np.asarray(conv_w, dtype=np.float32)
    conv_b = np.asarray(conv_b, dtype=np.float32)
    bn_scale = np.asarray(bn_scale, dtype=np.float32)
    bn_bias = np.asarray(bn_bias, dtype=np.float32)
    bn_mean = np.asarray(bn_mean, dtype=np.float32)
    bn_var = np.asarray(bn_var, dtype=np.float32)

    # host-side weight/param folding
    Wm = conv_w[:, :, 0, 0]                      # [384, 768]
    W1, W2 = Wm[:, :C], Wm[:, C:]
    wT = np.concatenate([(W1 + W2).T, (-W2).T], axis=0)   # [768, 384], rows = K
    wd_arr = np.ascontiguousarray(
        wT.reshape(2 * CC, 128, OC * 128).astype(ml_dtypes.bfloat16)
    )
    inv = 1.0 / np.sqrt(bn_var + 1e-5)
    a = (inv * bn_scale).astype(np.float32)               # per-channel scale
    b_aff = ((conv_b - bn_mean) * a + bn_bias).astype(np.float32)
    prm_arr = np.ascontiguousarray(
        np.stack([a.reshape(OC, 128), b_aff.reshape(OC, 128)], axis=-1)
    )                                                      # [3, 128, 2]

    xs = x.reshape(B, CC, 128, HW).astype(ml_dtypes.bfloat16)
    in_maps = []
    for core in range(N_CORES):
        shard = np.ascontiguousarray(xs[core * B_LOC:(core + 1) * B_LOC])
        in_maps.append({"x_in": shard, "w_in": wd_arr, "p_in": prm_arr})

    nc = _get_program()
    res = run_bass_kernel_spmd(nc, in_maps, core_ids=list(range(N_CORES)),
                               trace=_trace)
    LAST_RESULTS = res
    y = np.concatenate([r["y_out"] for r in res.results], axis=0)
    return y.reshape(B, C, H, W).astype(np.float32)


# revision 14
# speedup vs baseline: 1.0094x; 1.0094x over previous
"""Trainium2 Bass kernel for nn_Log_GraphConv4d (log-shift-max + 1x1 conv + BN + GeLU).

Math refactor (validated in numpy):
  reference x_j = max_s max(0, x - roll(x, s)) over s in {±1,±3,±7,±15,±31} on H and W
            == x - m,  where m = min(x, all 20 rolls)
  y = W1 @ x + W2 @ x_j            (1x1 conv, channel concat [x, x_j], K=768)
    = (W1+W2) @ x + (-W2) @ m
  BN (eval) + conv bias fold into per-out-channel affine (a, b):
  out = gelu(a * (Wc @ [x; m]) + b)

The min over the 20 rolls + x is computed with a 14-op binary min DAG
(proven minimal by exhaustive search over union-of-translates DAGs: each
axis needs build-ops + merge-leaves >= 7). All min ops run on DVE in bf16
2x mode; a helper copy X1 = roll(x, +1 in w) keeps W-axis reads 4B-aligned.

v2 changes vs baseline:
  - x is cast to bf16 on the host: input DMA traffic and fill latency halve.
  - X1 is built by GPSIMD tensor_copy (Pool engine is otherwise idle),
    freeing the Scalar engine for activations only.
  - y is written as bf16 and upcast on the host: drain shortens.
  - The final DAG op (min with x) is emitted per-C-chunk into 3 separate
    tiles so the K-contraction matmuls can start before the whole m tile
    is finished.

Sharding: data-parallel over batch, 2 samples per core across 8 cores;
weights replicated. No collectives.
"""

import numpy as np
import ml_dtypes

import concourse.bass as bass
import concourse.mybir as mybir
from concourse import bacc
from concourse.tile import TileContext
from concourse.bass_utils import run_bass_kernel_spmd

N_CORES = 8
B, C, H, W = 16, 384, 56, 56
HW = H * W                 # 3136
B_LOC = B // N_CORES       # 2 samples per core
CC = C // 128              # 3 input-channel chunks
OC = 384 // 128            # 3 output-channel chunks
NT = 7                     # n tiles over HW
NF = HW // NT              # 448 columns per matmul (fits one PSUM bank)

BF16 = mybir.dt.bfloat16
F32 = mybir.dt.float32
MIN = mybir.AluOpType.min
GELU = mybir.ActivationFunctionType.Gelu

LAST_RESULTS = None        # BassKernelResults of the most recent run (for test harness)


def _emit_min(eng, out, a, sa, b, sb, axis, L):
    """out = min(roll(a, sa, axis), roll(b, sb, axis)) on [128, cc, H, W] tiles.

    roll(t, s, axis)[i] = t[(i + s) mod L] along `axis` (2 = h, 3 = w).
    Circular wrap is handled by splitting into contiguous AP regions.
    """
    sa %= L
    sb %= L
    cuts = sorted({0, (L - sa) % L, (L - sb) % L})
    for idx, p in enumerate(cuts):
        q = cuts[idx + 1] if idx + 1 < len(cuts) else L
        n = q - p
        alo = (p + sa) % L
        blo = (p + sb) % L
        if axis == 3:
            eng.tensor_tensor(
                out=out[:, :, :, p:q],
                in0=a[:, :, :, alo:alo + n],
                in1=b[:, :, :, blo:blo + n],
                op=MIN,
            )
        else:
            eng.tensor_tensor(
                out=out[:, :, p:q, :],
                in0=a[:, :, alo:alo + n, :],
                in1=b[:, :, blo:blo + n, :],
                op=MIN,
            )


def _build(b_loc=B_LOC, cc=CC, oc=OC, act=GELU, repeat=1):
    nc = bacc.Bacc(None, target_bir_lowering=False)
    kc = 2 * cc

    xd = nc.dram_tensor("x_in", [b_loc, cc, 128, HW], BF16, kind="ExternalInput")
    wd = nc.dram_tensor("w_in", [kc, 128, oc * 128], BF16, kind="ExternalInput")
    pd = nc.dram_tensor("p_in", [oc, 128, 2], F32, kind="ExternalInput")
    yd = nc.dram_tensor("y_out", [b_loc, oc, 128, HW], BF16, kind="ExternalOutput")

    with TileContext(nc) as tc:
        with tc.tile_pool(name="sb", bufs=1) as pool, \
             tc.tile_pool(name="ps", bufs=6, space="PSUM") as psum:

            # --- prologue: weights + folded BN params (replicated, tiny) ---
            w_sb = pool.tile([128, kc, oc * 128], BF16, tag="w", bufs=1, name="w_sb")
            for k in range(kc):
                nc.sync.dma_start(out=w_sb[:, k, :], in_=wd[k])
            prm = pool.tile([128, oc, 2], F32, tag="prm", bufs=1, name="prm")
            nc.sync.dma_start(out=prm, in_=pd.rearrange("o p t -> p o t"))

            for b in [b for _ in range(repeat) for b in range(b_loc)]:
                last = (b == b_loc - 1)
                # --- load bf16 x (cast on host), one DMA per C-chunk ---
                X = pool.tile([128, cc, H, W], BF16, tag="X", bufs=2, name="X")
                Xf3 = X.rearrange("p c h w -> p c (h w)")
                for c in range(cc):
                    nc.sync.dma_start(out=Xf3[:, c], in_=xd[b, c])

                # X1 = roll(X, +1 in w) on Scalar, per C-chunk so it starts
                # as soon as each X chunk lands: keeps W-axis reads aligned.
                X1 = pool.tile([128, cc, H, W], BF16, tag="X1", bufs=1, name="X1")
                for c in range(cc):
                    nc.scalar.copy(out=X1[:, c, :, 0:W - 1], in_=X[:, c, :, 1:W])
                    nc.scalar.copy(out=X1[:, c, :, W - 1:W], in_=X[:, c, :, 0:1])

                # --- 14-op min DAG (offsets validated against reference) ---
                # H-chain (axis=2) on X directly (row stride is even):
                #   A2={±3} B2={1,±3,7} C2={±1,±3,±7} U2={15,31} V2={±15,±31}
                # W-chain (axis=3) via X1 so every shift is even.
                # Merges: M1=min(C,V) M2=min(C2,V2) M3=min(M1,M2) m=min(M3,X);
                # M2/M3 write in place.
                #
                # The channel chunks are fully independent, so the DAG can be
                # emitted whole-tile (lower dispatch overhead) or per-chunk
                # (m[c] lands early, letting the drain overlap). The last
                # sample uses per-chunk order to cut the pipeline tail; the
                # others use whole-tile with chunked A2/U2 heads for fill.
                tA2 = pool.tile([128, cc, H, W], BF16, tag="t4", bufs=1, name="tA2")
                tU2 = pool.tile([128, cc, H, W], BF16, tag="t5", bufs=1, name="tU2")
                tA = pool.tile([128, cc, H, W], BF16, tag="t1", bufs=1, name="tA")
                tB = pool.tile([128, cc, H, W], BF16, tag="t2", bufs=1, name="tB")
                tC = pool.tile([128, cc, H, W], BF16, tag="t3", bufs=1, name="tC")
                mpc = [pool.tile([128, H, W], BF16, tag=f"mp{c}", bufs=2,
                                 name=f"mp{c}") for c in range(cc)]

                def emit_chunk(cs, ce):
                    """Emit the full DAG restricted to chunks [cs:ce)."""
                    s = (slice(None), slice(cs, ce))
                    Xc, X1c = X[s], X1[s]
                    A2c, U2c, Ac, Bc, Cc = tA2[s], tU2[s], tA[s], tB[s], tC[s]
                    _emit_min(nc.vector, A2c, Xc, 3, Xc, -3, 2, H)     # A2
                    _emit_min(nc.vector, U2c, Xc, 15, Xc, 31, 2, H)    # U2
                    _emit_min(nc.vector, Ac, X1c, 2, X1c, -4, 3, W)    # A
                    _emit_min(nc.vector, Bc, Ac, 0, Ac, 4, 3, W)       # B
                    # B2 overwrites A (t1): A dead after B
                    _emit_min(nc.vector, Ac, A2c, 0, A2c, 4, 2, H)     # B2
                    _emit_min(nc.vector, Cc, Bc, 0, Bc, -4, 3, W)      # C
                    # U overwrites B (t2)
                    _emit_min(nc.vector, Bc, X1c, 14, X1c, 30, 3, W)   # U
                    # C2 overwrites A2 (t4), reading B2 (t1)
                    _emit_min(nc.vector, A2c, Ac, 0, Ac, -4, 2, H)     # C2
                    # V overwrites B2 (t1), reading U (t2)
                    _emit_min(nc.vector, Ac, Bc, 0, Bc, -46, 3, W)     # V
                    # V2 overwrites U (t2), reading U2 (t5)
                    _emit_min(nc.vector, Bc, U2c, 0, U2c, -46, 2, H)   # V2
                    for c in range(cs, ce):
                        # M1 = min(C, V) -> t5 (U2 dead after V2)
                        nc.vector.tensor_tensor(
                            out=tU2[:, c], in0=tC[:, c], in1=tA[:, c], op=MIN)
                        # M2 = min(C2, V2) -> t4 in place
                        nc.vector.tensor_tensor(
                            out=tA2[:, c], in0=tA2[:, c], in1=tB[:, c], op=MIN)
                        # M3 = min(M1, M2) -> t5 in place
                        nc.vector.tensor_tensor(
                            out=tU2[:, c], in0=tU2[:, c], in1=tA2[:, c], op=MIN)
                        # m = min(M3, X)
                        nc.vector.tensor_tensor(
                            out=mpc[c], in0=tU2[:, c], in1=X[:, c], op=MIN)

                if last:
                    for c in range(cc):
                        emit_chunk(c, c + 1)
                else:
                    # chunked A2/U2 heads: start on the first-landed X chunk
                    for c in range(cc):
                        _emit_min(nc.vector, tA2[:, c:c + 1], X[:, c:c + 1], 3,
                                  X[:, c:c + 1], -3, 2, H)
                    for c in range(cc):
                        _emit_min(nc.vector, tU2[:, c:c + 1], X[:, c:c + 1], 15,
                                  X[:, c:c + 1], 31, 2, H)
                    s = (slice(None), slice(0, cc))
                    Xc, X1c = X[s], X1[s]
                    A2c, U2c, Ac, Bc, Cc = tA2[s], tU2[s], tA[s], tB[s], tC[s]
                    _emit_min(nc.vector, Ac, X1c, 2, X1c, -4, 3, W)    # A
                    _emit_min(nc.vector, Bc, Ac, 0, Ac, 4, 3, W)       # B
                    _emit_min(nc.vector, Ac, A2c, 0, A2c, 4, 2, H)     # B2
                    _emit_min(nc.vector, Cc, Bc, 0, Bc, -4, 3, W)      # C
                    _emit_min(nc.vector, Bc, X1c, 14, X1c, 30, 3, W)   # U
                    _emit_min(nc.vector, A2c, Ac, 0, Ac, -4, 2, H)     # C2
                    _emit_min(nc.vector, Ac, Bc, 0, Bc, -46, 3, W)     # V
                    _emit_min(nc.vector, Bc, U2c, 0, U2c, -46, 2, H)   # V2
                    for c in range(cc):
                        nc.vector.tensor_tensor(
                            out=tU2[:, c], in0=tC[:, c], in1=tA[:, c], op=MIN)
                        nc.vector.tensor_tensor(
                            out=tA2[:, c], in0=tA2[:, c], in1=tB[:, c], op=MIN)
                        nc.vector.tensor_tensor(
                            out=tU2[:, c], in0=tU2[:, c], in1=tA2[:, c], op=MIN)
                        nc.vector.tensor_tensor(
                            out=mpc[c], in0=tU2[:, c], in1=X[:, c], op=MIN)

                # --- matmul K=2C contraction + fused BN-affine + GeLU ---
                Xf = X.rearrange("p c h w -> p c (h w)")
                Mfs = [m.rearrange("p h w -> p (h w)") for m in mpc]
                # k-phase emission with 7 PSUM tiles per o: all the
                # X-dependent accumulation (k<cc) issues before any
                # m-dependent matmul, so the PE queue never head-of-line
                # blocks on mp chunks mid-tile.
                for o in range(oc):
                    y_sb = pool.tile([128, HW], BF16, tag="y", bufs=2, name="y_sb")
                    psts = [psum.tile([128, NF], F32, tag="ps", bufs=7,
                                      name=f"pst{n}") for n in range(NT)]
                    for k in range(kc):
                        for n in range(NT):
                            if k < cc:
                                src = Xf[:, k, n * NF:(n + 1) * NF]
                            else:
                                src = Mfs[k - cc][:, n * NF:(n + 1) * NF]
                            nc.tensor.matmul(
                                psts[n],
                                lhsT=w_sb[:, k, o * 128:(o + 1) * 128],
                                rhs=src,
                                start=(k == 0),
                                stop=(k == kc - 1),
                            )
                    for n in range(NT):
                        nc.scalar.activation(
                            out=y_sb[:, n * NF:(n + 1) * NF],
                            in_=psts[n],
                            func=act,
                            bias=prm[:, o, 1:2],
                            scale=prm[:, o, 0:1],
                        )
                        nc.sync.dma_start(
                            out=yd[b, o, :, n * NF:(n + 1) * NF],
                            in_=y_sb[:, n * NF:(n + 1) * NF],
                        )
    nc.finalize()  # Bacc: wait-splitting, reg alloc, event sems — required by walrus
    return nc


_CACHE = {}


def _get_program():
    if "nc" not in _CACHE:
        _CACHE["nc"] = _build()
    return _CACHE["nc"]


def kernel(x, conv_w, conv_b, bn_scale, bn_bias, bn_mean, bn_var, _trace=False):
    global LAST_RESULTS
    x = np.asarray(x, dtype=np.float32)
    conv_w = np.asarray(conv_w, dtype=np.float32)
    conv_b = np.asarray(conv_b, dtype=np.float32)
    bn_scale = np.asarray(bn_scale, dtype=np.float32)
    bn_bias = np.asarray(bn_bias, dtype=np.float32)
    bn_mean = np.asarray(bn_mean, dtype=np.float32)
    bn_var = np.asarray(bn_var, dtype=np.float32)

    # host-side weight/param folding
    Wm = conv_w[:, :, 0, 0]                      # [384, 768]
    W1, W2 = Wm[:, :C], Wm[:, C:]
    wT = np.concatenate([(W1 + W2).T, (-W2).T], axis=0)   # [768, 384], rows = K
    wd_arr = np.ascontiguousarray(
        wT.reshape(2 * CC, 128, OC * 128).astype(ml_dtypes.bfloat16)
    )
    inv = 1.0 / np.sqrt(bn_var + 1e-5)
    a = (inv * bn_scale).astype(np.float32)               # per-channel scale
    b_aff = ((conv_b - bn_mean) * a + bn_bias).astype(np.float32)
    prm_arr = np.ascontiguousarray(
        np.stack([a.reshape(OC, 128), b_aff.reshape(OC, 128)], axis=-1)
    )                                                      # [3, 128, 2]

    xs = x.reshape(B, CC, 128, HW).astype(ml_dtypes.bfloat16)
    in_maps = []
    for core in range(N_CORES):
        shard = np.ascontiguousarray(xs[core * B_LOC:(core + 1) * B_LOC])
        in_maps.append({"x_in": shard, "w_in": wd_arr, "p_in": prm_arr})

    nc = _get_program()
    res = run_bass_kernel_spmd(nc, in_maps, core_ids=list(range(N_CORES)),
                               trace=_trace)
    LAST_RESULTS = res
    y = np.concatenate([r["y_out"] for r in res.results], axis=0)
    return y.reshape(B, C, H, W).astype(np.float32)


# revision 18
# speedup vs baseline: 1.0828x; 1.0726x over previous
"""Trainium2 Bass kernel for nn_Log_GraphConv4d (log-shift-max + 1x1 conv + BN + GeLU).

Math refactor (validated in numpy):
  reference x_j = max_s max(0, x - roll(x, s)) over s in {±1,±3,±7,±15,±31} on H and W
            == x - m,  where m = min(x, all 20 rolls)
  y = W1 @ x + W2 @ x_j            (1x1 conv, channel concat [x, x_j], K=768)
    = (W1+W2) @ x + (-W2) @ m
  BN (eval) + conv bias fold into per-out-channel affine (a, b):
  out = gelu(a * (Wc @ [x; m]) + b)

The min over the 20 rolls + x is computed with a 14-op binary min DAG
(proven minimal by exhaustive search over union-of-translates DAGs: each
axis needs build-ops + merge-leaves >= 7). All min ops run on DVE in bf16
2x mode; a helper copy X1 = roll(x, +1 in w) keeps W-axis reads 4B-aligned.

v2 changes vs baseline:
  - x is cast to bf16 on the host: input DMA traffic and fill latency halve.
  - X1 is built by GPSIMD tensor_copy (Pool engine is otherwise idle),
    freeing the Scalar engine for activations only.
  - y is written as bf16 and upcast on the host: drain shortens.
  - The final DAG op (min with x) is emitted per-C-chunk into 3 separate
    tiles so the K-contraction matmuls can start before the whole m tile
    is finished.

Sharding: data-parallel over batch, 2 samples per core across 8 cores;
weights replicated. No collectives.
"""

import numpy as np
import ml_dtypes

import concourse.bass as bass
import concourse.mybir as mybir
from concourse import bacc
from concourse.tile import TileContext
from concourse.bass_utils import run_bass_kernel_spmd

N_CORES = 8
B, C, H, W = 16, 384, 56, 56
HW = H * W                 # 3136
B_LOC = B // N_CORES       # 2 samples per core
CC = C // 128              # 3 input-channel chunks
OC = 384 // 128            # 3 output-channel chunks
NT = 7                     # n tiles over HW
NF = HW // NT              # 448 columns per matmul (fits one PSUM bank)

BF16 = mybir.dt.bfloat16
F32 = mybir.dt.float32
MIN = mybir.AluOpType.min
GELU = mybir.ActivationFunctionType.Gelu

LAST_RESULTS = None        # BassKernelResults of the most recent run (for test harness)


def _emit_min(eng, out, a, sa, b, sb, axis, L):
    """out = min(roll(a, sa, axis), roll(b, sb, axis)) on [128, cc, H, W] tiles.

    roll(t, s, axis)[i] = t[(i + s) mod L] along `axis` (2 = h, 3 = w).
    Circular wrap is handled by splitting into contiguous AP regions.
    """
    sa %= L
    sb %= L
    cuts = sorted({0, (L - sa) % L, (L - sb) % L})
    for idx, p in enumerate(cuts):
        q = cuts[idx + 1] if idx + 1 < len(cuts) else L
        n = q - p
        alo = (p + sa) % L
        blo = (p + sb) % L
        if axis == 3:
            eng.tensor_tensor(
                out=out[:, :, :, p:q],
                in0=a[:, :, :, alo:alo + n],
                in1=b[:, :, :, blo:blo + n],
                op=MIN,
            )
        else:
            eng.tensor_tensor(
                out=out[:, :, p:q, :],
                in0=a[:, :, alo:alo + n, :],
                in1=b[:, :, blo:blo + n, :],
                op=MIN,
            )


def _build(b_loc=B_LOC, cc=CC, oc=OC, act=GELU, repeat=1):
    nc = bacc.Bacc(None, target_bir_lowering=False)
    kc = 2 * cc

    xd = nc.dram_tensor("x_in", [b_loc, cc, 128, HW], BF16, kind="ExternalInput")
    wd = nc.dram_tensor("w_in", [kc, 128, oc * 128], BF16, kind="ExternalInput")
    pd = nc.dram_tensor("p_in", [oc, 128, 2], F32, kind="ExternalInput")
    yd = nc.dram_tensor("y_out", [b_loc, oc, 128, HW], BF16, kind="ExternalOutput")

    with TileContext(nc) as tc:
        with tc.tile_pool(name="sb", bufs=1) as pool, \
             tc.tile_pool(name="ps", bufs=6, space="PSUM") as psum:

            # --- prologue: weights + folded BN params (replicated, tiny) ---
            w_sb = pool.tile([128, kc, oc * 128], BF16, tag="w", bufs=1, name="w_sb")
            for k in range(kc):
                nc.sync.dma_start(out=w_sb[:, k, :], in_=wd[k])
            prm = pool.tile([128, oc, 2], F32, tag="prm", bufs=1, name="prm")
            nc.sync.dma_start(out=prm, in_=pd.rearrange("o p t -> p o t"))

            for b in [b for _ in range(repeat) for b in range(b_loc)]:
                last = (b == b_loc - 1)
                # --- load bf16 x (cast on host), one DMA per C-chunk ---
                X = pool.tile([128, cc, H, W], BF16, tag="X", bufs=2, name="X")
                Xf3 = X.rearrange("p c h w -> p c (h w)")
                for c in range(cc):
                    nc.sync.dma_start(out=Xf3[:, c], in_=xd[b, c])

                X1 = pool.tile([128, cc, H, W], BF16, tag="X1", bufs=1, name="X1")

                # --- 14-op min DAG (offsets validated against reference) ---
                # H-chain (axis=2) on X directly (row stride is even):
                #   A2={±3} B2={1,±3,7} C2={±1,±3,±7} U2={15,31} V2={±15,±31}
                # W-chain (axis=3) via X1 so every shift is even.
                # Merges: M1=min(C,V) M2=min(C2,V2) M3=min(M1,M2) m=min(M3,X);
                # M2/M3 write in place.
                #
                # Register-style tile reuse (5 temps): A2->t4 U2->t5 A->t1
                # B->t2 C->t3, then B2 overwrites t1, U->t2, C2->t4, V->t1,
                # V2->t2, M1->t5, M2->t4 (in place), M3->t5 (in place).
                tA2 = pool.tile([128, cc, H, W], BF16, tag="t4", bufs=1, name="tA2")
                tU2 = pool.tile([128, cc, H, W], BF16, tag="t5", bufs=1, name="tU2")
                tA = pool.tile([128, cc, H, W], BF16, tag="t1", bufs=1, name="tA")
                tB = pool.tile([128, cc, H, W], BF16, tag="t2", bufs=1, name="tB")
                tC = pool.tile([128, cc, H, W], BF16, tag="t3", bufs=1, name="tC")
                mpc = [pool.tile([128, H, W], BF16, tag=f"mp{c}", bufs=2,
                                 name=f"mp{c}") for c in range(cc)]

                # chunked A2/U2 heads: start on the first-landed X chunk
                for c in range(cc):
                    _emit_min(nc.vector, tA2[:, c:c + 1], X[:, c:c + 1], 3,
                              X[:, c:c + 1], -3, 2, H)
                for c in range(cc):
                    _emit_min(nc.vector, tU2[:, c:c + 1], X[:, c:c + 1], 15,
                              X[:, c:c + 1], 31, 2, H)
                # X1 copies emitted after the H-op heads (scalar engine)
                for c in range(cc):
                    nc.scalar.copy(out=X1[:, c, :, 0:W - 1], in_=X[:, c, :, 1:W])
                    nc.scalar.copy(out=X1[:, c, :, W - 1:W], in_=X[:, c, :, 0:1])
                _emit_min(nc.vector, tA, X1, 2, X1, -4, 3, W)      # A
                _emit_min(nc.vector, tB, tA, 0, tA, 4, 3, W)       # B
                _emit_min(nc.vector, tA, tA2, 0, tA2, 4, 2, H)     # B2
                _emit_min(nc.vector, tC, tB, 0, tB, -4, 3, W)      # C
                _emit_min(nc.vector, tB, X1, 14, X1, 30, 3, W)     # U
                _emit_min(nc.vector, tA2, tA, 0, tA, -4, 2, H)     # C2
                _emit_min(nc.vector, tA, tB, 0, tB, -46, 3, W)     # V
                _emit_min(nc.vector, tB, tU2, 0, tU2, -46, 2, H)   # V2
                for c in range(cc):
                    nc.vector.tensor_tensor(
                        out=tU2[:, c], in0=tC[:, c], in1=tA[:, c], op=MIN)
                    nc.vector.tensor_tensor(
                        out=tA2[:, c], in0=tA2[:, c], in1=tB[:, c], op=MIN)
                    nc.vector.tensor_tensor(
                        out=tU2[:, c], in0=tU2[:, c], in1=tA2[:, c], op=MIN)
                    nc.vector.tensor_tensor(
                        out=mpc[c], in0=tU2[:, c], in1=X[:, c], op=MIN)

                # --- matmul K=2C contraction + fused BN-affine + GeLU ---
                Xf = X.rearrange("p c h w -> p c (h w)")
                Mfs = [m.rearrange("p h w -> p (h w)") for m in mpc]
                for o in range(oc):
                    y_sb = pool.tile([128, HW], BF16, tag="y", bufs=2, name="y_sb")
                    for n in range(NT):
                        pst = psum.tile([128, NF], F32, tag="ps", bufs=8,
                                        name="pst")
                        for k in range(kc):
                            if k < cc:
                                src = Xf[:, k, n * NF:(n + 1) * NF]
                            else:
                                src = Mfs[k - cc][:, n * NF:(n + 1) * NF]
                            nc.tensor.matmul(
                                pst,
                                lhsT=w_sb[:, k, o * 128:(o + 1) * 128],
                                rhs=src,
                                start=(k == 0),
                                stop=(k == kc - 1),
                            )
                        nc.scalar.activation(
                            out=y_sb[:, n * NF:(n + 1) * NF],
                            in_=pst,
                            func=act,
                            bias=prm[:, o, 1:2],
                            scale=prm[:, o, 0:1],
                        )
                        nc.sync.dma_start(
                            out=yd[b, o, :, n * NF:(n + 1) * NF],
                            in_=y_sb[:, n * NF:(n + 1) * NF],
                        )
    nc.finalize()  # Bacc: wait-splitting, reg alloc, event sems — required by walrus
    return nc


_CACHE = {}


def _get_program():
    if "nc" not in _CACHE:
        _CACHE["nc"] = _build()
    return _CACHE["nc"]


def kernel(x, conv_w, conv_b, bn_scale, bn_bias, bn_mean, bn_var, _trace=False):
    global LAST_RESULTS
    x = np.asarray(x, dtype=np.float32)
    conv_w = np.asarray(conv_w, dtype=np.float32)
    conv_b = np.asarray(conv_b, dtype=np.float32)
    bn_scale = np.asarray(bn_scale, dtype=np.float32)
    bn_bias = np.asarray(bn_bias, dtype=np.float32)
    bn_mean = np.asarray(bn_mean, dtype=np.float32)
    bn_var = np.asarray(bn_var, dtype=np.float32)

    # host-side weight/param folding
    Wm = conv_w[:, :, 0, 0]                      # [384, 768]
    W1, W2 = Wm[:, :C], Wm[:, C:]
    wT = np.concatenate([(W1 + W2).T, (-W2).T], axis=0)   # [768, 384], rows = K
    wd_arr = np.ascontiguousarray(
        wT.reshape(2 * CC, 128, OC * 128).astype(ml_dtypes.bfloat16)
    )
    inv = 1.0 / np.sqrt(bn_var + 1e-5)
    a = (inv * bn_scale).astype(np.float32)               # per-channel scale
    b_aff = ((conv_b - bn_mean) * a + bn_bias).astype(np.float32)
    prm_arr = np.ascontiguousarray(
        np.stack([a.reshape(OC, 128), b_aff.reshape(OC, 128)], axis=-1)
    )                                                      # [3, 128, 2]

    xs = x.reshape(B, CC, 128, HW).astype(ml_dtypes.bfloat16)
    in_maps = []
    for core in range(N_CORES):
        shard = np.ascontiguousarray(xs[core * B_LOC:(core + 1) * B_LOC])
        in_maps.append({"x_in": shard, "w_in": wd_arr, "p_in": prm_arr})

    nc = _get_program()
    res = run_bass_kernel_spmd(nc, in_maps, core_ids=list(range(N_CORES)),
                               trace=_trace)
    LAST_RESULTS = res
    y = np.concatenate([r["y_out"] for r in res.results], axis=0)
    return y.reshape(B, C, H, W).astype(np.float32)
